# revision 15
# baseline (speedup 1.0000x reference)
"""Causal self-attention on 8 Trainium2 NeuronCores.

Problem: x[4,2048,1024], Wq/Wk/Wv/Wo[1024,1024], H=16 heads, dh=64.
    q,k,v = x@W{q,k,v}.T ; per-head causal softmax(q k^T/8) v ; out = y@Wo.T

Sharding (hybrid data+tensor parallel over 8 cores):
  core c -> (batch b = c//2, head-group hg = c%2 of 8 heads = 512 dims).
  Each core computes a partial output out_c[b] = y_hg @ Wo[:, hg].T ; the
  host sums the two bf16 partials per batch in f32 (the Wo all-reduce).

Per-core kernel dataflow (layouts avoid any on-device transposes):
  xT[1024,2048] (=x[b].T, loaded whole via 8 full-row DMAs),
  wqkT[1024,1024] (=[Wq[hg].T | Wk[hg].T] packed for 2KB DMA lines),
  wvT[1024,512], woT[512,1024] (=Wo[:,hg].T)
  stage 1 (per 512-col t-tile): QT[j,t], KT[j,t] via matmul(lhsT=w, rhs=xT);
           V[t,i] via matmul(lhsT=xT slice, rhs=wvT); a ones column per head
           appended to V so the PV matmul emits softmax row-sums for free.
  stage 2: per head pair g, per 128-wide k-tile: the two heads' S^T[k,q]
           matmuls are row-tiled on the PE array (K=64 at tile rows 0/64,
           concurrent) into separate PSUM banks; exp(S/8) runs per head on
           ACT (bf16 out) so the PV of head A overlaps the exp of head B;
           causal handling: all ops on diagonal k-tiles trimmed to
           q >= 128m plus one [128,128] 0/1 triangle-mask multiply on the
           diagonal block; yT_h[65,q] += V'_h[k,65]^T P^T_h (row 64 =
           softmax denominator). Normalize: rowsum -> SBUF -> fast recip ->
           bf16 -> K=1 ones-matmul broadcast to 64 partitions -> multiply.
  stage 3: outT[o,t] = matmul(lhsT=woT, rhs=yT) -> bf16 -> DMA out.

  QT/KT/V/yT are split per t-tile so the Tile list-scheduler can overlap
  stage-1 and stage-3 matmuls with the latency-bound attention loop;
  emission interleaves s1(t+1) chunks into s2(qi<3) and all of s3(t0..t2)
  into s2(qi=3) to keep the PE dense (HAM-warm) throughout.

Precision: matmul operands bf16, PSUM accumulation fp32, softmax
reciprocal via fast-approx (f32, SBUF-staged: the custom DVE op reading
PSUM directly is silently wrong on HW) cast to bf16. exp needs no
max-subtraction: att ~ N(0,1) here, |att| < ~7, exp is safe in fp32.
"""

import sys

import numpy as np

sys.path.insert(0, "/opt/trn_rl_repo")

import concourse.bass as bass  # noqa: F401
from concourse import bacc
import concourse.mybir as mybir
import concourse.tile as tile
from concourse.bass_utils import run_bass_kernel_spmd

B, T, D, H, DH = 4, 2048, 1024, 16, 64
NCORES = 8
HPC = 8                 # heads per core
JJ = HPC * DH           # 512: per-core qkv head dims
P = 128
TQ = 512                # attention q tile (free dim of S^T matmul)
TK = 128                # attention k tile (partition dim of S^T)
NDT = D // P            # 8 d-tiles (contraction for stage 1)
NJT = JJ // P           # 4 j-tiles (head-pair tiles)
NTT = T // TQ           # 4 t-tiles of 512
NKT = T // TK           # 16 k-tiles of 128
NOT_ = D // P           # 8 output row tiles (stage 3)
VW = 66                 # V row width: 64 dh + 1 ones + 1 pad
F32 = mybir.dt.float32
BF16 = mybir.dt.bfloat16


def build_program():
    nc = bacc.Bacc()
    xT = nc.dram_tensor("xT", [D, T], BF16, kind="ExternalInput")
    wqkT = nc.dram_tensor("wqkT", [D, 2 * JJ], BF16, kind="ExternalInput")
    wvT = nc.dram_tensor("wvT", [D, JJ], BF16, kind="ExternalInput")
    woT = nc.dram_tensor("woT", [JJ, D], BF16, kind="ExternalInput")
    maskd = nc.dram_tensor("mask", [P, P], BF16, kind="ExternalInput")
    outT = nc.dram_tensor("outT", [D, T], BF16, kind="ExternalOutput")

    xTv = xT.rearrange("(n p) t -> n p t", p=P)        # [8,128,2048]
    wqkv = wqkT.rearrange("(n p) j -> n p j", p=P)     # [8,128,1024]
    wvv = wvT.rearrange("(n p) j -> n p j", p=P)       # [8,128,512]
    wov = woT.rearrange("(n p) o -> n p o", p=P)       # [4,128,1024]
    outv = outT.rearrange("(n p) t -> n p t", p=P)     # [8,128,2048]

    inv8 = 1.0 / float(np.sqrt(DH))

    with tile.TileContext(nc) as tc:
        with (
            tc.tile_pool(name="persist", bufs=1) as persist,
            tc.tile_pool(name="ptpool", bufs=6) as ptpool,
            tc.tile_pool(name="small", bufs=1) as small,
            tc.tile_pool(name="psS", bufs=1, space="PSUM") as psS,
            tc.tile_pool(name="psY", bufs=1, space="PSUM") as psY,
        ):
            # ---- persistent SBUF tensors ----
            x_sb = persist.tile([P, NDT, T], BF16)        # whole xT
            qt = [persist.tile([P, NJT, TQ], BF16, name=f"qt{i}")
                  for i in range(NTT)]
            kt = [persist.tile([P, NJT, TQ], BF16, name=f"kt{i}")
                  for i in range(NTT)]
            vt = [persist.tile([P, TQ // P, HPC, VW], BF16, name=f"vt{i}")
                  for i in range(NTT)]
            yt = [persist.tile([P, NJT, TQ], BF16, name=f"yt{i}")
                  for i in range(NTT)]
            mask_sb = persist.tile([P, P], BF16)
            ones_bf = persist.tile([1, P], BF16)
            wqk_sb = persist.tile([P, NDT, 2 * JJ], BF16)
            wv_sb = persist.tile([P, NDT, JJ], BF16)
            wo_sb = persist.tile([P, NJT, D], BF16)

            for i in range(NTT):
                nc.any.memset(vt[i][:, :, :, DH : DH + 1], 1.0)
            nc.any.memset(ones_bf[:], 1.0)

            # ---- input DMAs, in consumption order ----
            nc.sync.dma_start(out=mask_sb[:], in_=maskd[:, :])
            for dt_ in range(NDT):
                nc.sync.dma_start(out=wqk_sb[:, dt_, :], in_=wqkv[dt_])
            for dt_ in range(NDT):
                nc.sync.dma_start(out=x_sb[:, dt_, :], in_=xTv[dt_])
            for dt_ in range(NDT):
                nc.sync.dma_start(out=wv_sb[:, dt_, :], in_=wvv[dt_])
            for it in range(NJT):
                nc.sync.dma_start(out=wo_sb[:, it, :], in_=wov[it])

            # ---- stage 1 chunks: QKV projection for one t-tile ----
            # 12 groups per ti: (q,jt0..3),(k,jt0..3),(v,ts0..3)
            def s1_group(ti, gi):
                tsl = slice(ti * TQ, (ti + 1) * TQ)
                if gi < 8:
                    o_sb = qt[ti] if gi < 4 else kt[ti]
                    jt = gi % 4
                    joff = (0 if gi < 4 else JJ) + jt * P
                    ps = psS.tile([P, TQ], F32, tag="mm", bufs=2)
                    for dt_ in range(NDT):
                        nc.tensor.matmul(
                            ps[:],
                            lhsT=wqk_sb[:, dt_, joff : joff + P],
                            rhs=x_sb[:, dt_, tsl],
                            start=(dt_ == 0),
                            stop=(dt_ == NDT - 1),
                        )
                    nc.vector.tensor_copy(o_sb[:, jt, :], ps[:])
                else:
                    tsub = gi - 8
                    ssl = slice(ti * TQ + tsub * P, ti * TQ + (tsub + 1) * P)
                    ps = psS.tile([P, JJ], F32, tag="mm", bufs=2)
                    for dt_ in range(NDT):
                        nc.tensor.matmul(
                            ps[:],
                            lhsT=x_sb[:, dt_, ssl],
                            rhs=wv_sb[:, dt_, :],
                            start=(dt_ == 0),
                            stop=(dt_ == NDT - 1),
                        )
                    nc.vector.tensor_copy(
                        vt[ti][:, tsub, :, 0:DH],
                        ps[:].rearrange("p (h i) -> p h i", h=HPC),
                    )

            # ---- stage 3 chunks: output projection for one (ti, ot) ----
            def s3_group(ti, ot):
                tsl = slice(ti * TQ, (ti + 1) * TQ)
                osl = slice(ot * P, (ot + 1) * P)
                ps = psS.tile([P, TQ], F32, tag="mm", bufs=2)
                for it in range(NJT):
                    nc.tensor.matmul(
                        ps[:],
                        lhsT=wo_sb[:, it, osl],
                        rhs=yt[ti][:, it, :],
                        start=(it == 0),
                        stop=(it == NJT - 1),
                    )
                o_sb = small.tile([P, TQ], BF16, tag="ostage", bufs=3)
                nc.vector.tensor_copy(o_sb[:], ps[:])
                nc.sync.dma_start(out=outv[ot][:, tsl], in_=o_sb[:])

            # ---- stage 2: attention for one (qi, g) head pair ----
            def s2_block(qi, g):
                n_full = 4 * qi
                nkt = n_full + 4
                y2 = psY.tile([DH + 1, 2, TQ], F32, tag="y", bufs=2,
                              name=f"y2_{qi}_{g}")
                for kt_i in range(nkt):
                    m = kt_i - n_full
                    qoff = 0 if m < 0 else m * P
                    ksl = slice((kt_i % 4) * P, (kt_i % 4 + 1) * P)
                    for hh in range(2):
                        hsl = slice(hh * DH, (hh + 1) * DH)
                        s2 = psS.tile([P, TQ], F32, tag=f"att{hh}", bufs=1)
                        nc.tensor.matmul(
                            s2[:, qoff:TQ],
                            lhsT=kt[kt_i // 4][hsl, g, ksl],
                            rhs=qt[qi][hsl, g, qoff:TQ],
                            start=True,
                            stop=True,
                        )
                        pth = ptpool.tile([P, TQ], BF16, tag="pt")
                        nc.scalar.activation(
                            pth[:, qoff:TQ], s2[:, qoff:TQ],
                            mybir.ActivationFunctionType.Exp,
                            scale=inv8,
                        )
                        if m >= 0:  # diagonal block: zero where k > q
                            nc.vector.tensor_tensor(
                                pth[:, qoff : qoff + P],
                                pth[:, qoff : qoff + P],
                                mask_sb[:],
                                mybir.AluOpType.mult,
                            )
                        nc.tensor.matmul(
                            y2[:, hh, qoff:TQ],
                            lhsT=vt[kt_i // 4][:, kt_i % 4, 2 * g + hh,
                                              0 : DH + 1],
                            rhs=pth[:, qoff:TQ],
                            start=(kt_i == 0),
                            stop=(kt_i == nkt - 1),
                        )
                # normalize: 1/rowsum broadcast to 64 partitions via PE
                rs_sb = small.tile([1, 2, TQ], F32, tag="rs_sb", bufs=2)
                for hh in range(2):
                    nc.vector.tensor_copy(
                        rs_sb[0:1, hh, :], y2[DH : DH + 1, hh, :]
                    )
                recipf = small.tile([1, 2, TQ], F32, tag="recipf", bufs=2)
                nc.vector.reciprocal_approx_fast(recipf[:], rs_sb[:])
                rbf = small.tile([1, 2, TQ], BF16, tag="rbf", bufs=2)
                nc.vector.tensor_copy(rbf[:], recipf[:])
                for hh in range(2):
                    bc_ps = psS.tile([DH, TQ], F32, tag="mm", bufs=2)
                    nc.tensor.matmul(
                        bc_ps[:],
                        lhsT=ones_bf[0:1, 0:DH],
                        rhs=rbf[0:1, hh, :],
                        start=True,
                        stop=True,
                    )
                    bcb = small.tile([DH, TQ], BF16, tag="bcb", bufs=2)
                    nc.vector.tensor_copy(bcb[:], bc_ps[:])
                    nc.vector.tensor_tensor(
                        yt[qi][hh * DH : (hh + 1) * DH, g, :],
                        y2[0:DH, hh, :],
                        bcb[:],
                        mybir.AluOpType.mult,
                    )

            # ---- emission schedule ----
            for gi in range(12):
                s1_group(0, gi)
            for qi in range(NTT):
                for g in range(NJT):
                    s2_block(qi, g)
                    if qi + 1 < NTT:
                        # interleave next-ti stage1 (3 groups per g)
                        for gi in range(3 * g, 3 * g + 3):
                            s1_group(qi + 1, gi)
                    else:
                        # qi==3: fill with stage3 of t0..t2 (6 groups per g)
                        for j in range(6 * g, 6 * g + 6):
                            s3_group(j // 8, j % 8)
            for ot in range(NOT_):
                s3_group(NTT - 1, ot)

    nc.compile()
    return nc


def _make_mask():
    k = np.arange(P)[:, None]
    j = np.arange(P)[None, :]
    return (j >= k).astype(np.float32)


def make_in_maps(x, Wq, Wk, Wv, Wo):
    import ml_dtypes

    bf = ml_dtypes.bfloat16
    mask = _make_mask().astype(bf)
    x = np.asarray(x, np.float32)
    Wq, Wk, Wv, Wo = (np.asarray(w, np.float32) for w in (Wq, Wk, Wv, Wo))
    in_maps = []
    for c in range(NCORES):
        b, hg = c // 2, c % 2
        sl = slice(hg * JJ, (hg + 1) * JJ)
        wqk = np.concatenate(
            [np.ascontiguousarray(Wq[sl].T), np.ascontiguousarray(Wk[sl].T)],
            axis=1,
        )
        in_maps.append({
            "xT": np.ascontiguousarray(x[b].T).astype(bf),
            "wqkT": np.ascontiguousarray(wqk).astype(bf),
            "wvT": np.ascontiguousarray(Wv[sl].T).astype(bf),
            "woT": np.ascontiguousarray(Wo[:, sl].T).astype(bf),
            "mask": mask,
        })
    return in_maps


def gather_output(results):
    out = np.zeros((B, T, D), np.float32)
    for c in range(NCORES):
        out[c // 2] += results[c]["outT"].T.astype(np.float32)
    return out


def kernel(x, Wq, Wk, Wv, Wo):
    nc = build_program()
    in_maps = make_in_maps(x, Wq, Wk, Wv, Wo)
    res = run_bass_kernel_spmd(nc, in_maps, list(range(NCORES)))
    return gather_output(res.results)


if __name__ == "__main__":
    rng = np.random.default_rng(0)
    xs = [rng.standard_normal(s, dtype=np.float32) for s in
          [(B, T, D), (D, D), (D, D), (D, D), (D, D)]]
    out = kernel(*xs)
    print(out.shape, out.dtype)


# revision 17
# speedup vs baseline: 1.2381x; 1.2381x over previous
"""Causal self-attention on 8 Trainium2 NeuronCores.

Problem: x[4,2048,1024], Wq/Wk/Wv/Wo[1024,1024], H=16 heads, dh=64.
    q,k,v = x@W{q,k,v}.T ; per-head causal softmax(q k^T/8) v ; out = y@Wo.T

Sharding (hybrid data+tensor parallel over 8 cores):
  core c -> (batch b = c//2, head-group hg = c%2 of 8 heads = 512 dims).
  Each core computes a partial output out_c[b] = y_hg @ Wo[:, hg].T ; the
  host sums the two bf16 partials per batch in f32 (the Wo all-reduce).

Per-core kernel dataflow (layouts avoid any on-device transposes):
  xT[1024,2048] (=x[b].T, loaded whole via 8 full-row 4KB/partition DMAs),
  wqkT[1024,1024] (=[Wq[hg].T | Wk[hg].T] packed for 2KB DMA lines),
  wvT[1024,512], woT[512,1024] (=Wo[:,hg].T)
  stage 1 (per 512-col t-tile): QT[j,t], KT[j,t] via matmul(lhsT=w, rhs=xT);
           V[t,i] via matmul(lhsT=xT slice, rhs=wvT); a ones column per head
           appended to V so the PV matmul emits softmax row-sums for free.
  stage 2: per head pair g, per 128-wide k-tile: the two heads' S^T[k,q]
           matmuls row-tiled on the PE (K=64 at tile rows 0/64, concurrent)
           into adjacent PSUM banks; one fused exp(S/8) over both banks on
           ACT (bf16 out); causal handling: ops on diagonal k-tiles trimmed
           to q >= 128m plus one [128,128] triangle-mask multiply on the
           diagonal block; yT_h[65,q] += V'_h[k,65]^T P^T_h (row 64 =
           softmax denominator).
           The kt loop is software-pipelined in EMISSION order: PV(kt) is
           emitted after S(kt+1), and 1..6 stage-1/stage-3 filler matmuls
           (yielded one at a time by generators) are emitted per iteration
           so the in-order PE queue always has independent work while exp
           runs — keeps the PE dense and the HAM clock-gate warm.
           Normalize per g: rowsum row + unnormalized y copied out of PSUM
           early (frees the y bank for the next block), fast reciprocal ->
           bf16 -> K=1 ones-matmul broadcast to 64 partitions -> multiply.
  stage 3: outT[o,t] = matmul(lhsT=woT, rhs=yT) -> bf16 -> DMA out.

Precision: matmul operands bf16, PSUM accumulation fp32, softmax
reciprocal via fast-approx (f32, SBUF-staged: the custom DVE op reading
PSUM directly is silently wrong on HW) cast to bf16. exp needs no
max-subtraction: att ~ N(0,1) here, |att| < ~7, exp is safe in fp32.
"""

import sys

import numpy as np

sys.path.insert(0, "/opt/trn_rl_repo")

import concourse.bass as bass  # noqa: F401
from concourse import bacc
import concourse.mybir as mybir
import concourse.tile as tile
from concourse.bass_utils import run_bass_kernel_spmd

B, T, D, H, DH = 4, 2048, 1024, 16, 64
NCORES = 8
HPC = 8                 # heads per core
JJ = HPC * DH           # 512: per-core qkv head dims
P = 128
TQ = 512                # attention q tile (free dim of S^T matmul)
TK = 128                # attention k tile (partition dim of S^T)
NDT = D // P            # 8 d-tiles (contraction for stage 1)
NJT = JJ // P           # 4 j-tiles (head-pair tiles)
NTT = T // TQ           # 4 t-tiles of 512
NKT = T // TK           # 16 k-tiles of 128
NOT_ = D // P           # 8 output row tiles (stage 3)
VW = 66                 # V row width: 64 dh + 1 ones + 1 pad
F32 = mybir.dt.float32
BF16 = mybir.dt.bfloat16


def build_program():
    nc = bacc.Bacc()
    xT = nc.dram_tensor("xT", [D, T], BF16, kind="ExternalInput")
    wqkT = nc.dram_tensor("wqkT", [D, 2 * JJ], BF16, kind="ExternalInput")
    wvT = nc.dram_tensor("wvT", [D, JJ], BF16, kind="ExternalInput")
    woT = nc.dram_tensor("woT", [JJ, D], BF16, kind="ExternalInput")
    maskd = nc.dram_tensor("mask", [P, P], BF16, kind="ExternalInput")
    outT = nc.dram_tensor("outT", [D, T], BF16, kind="ExternalOutput")

    xTv = xT.rearrange("(n p) t -> n p t", p=P)        # [8,128,2048]
    wqkv = wqkT.rearrange("(n p) j -> n p j", p=P)     # [8,128,1024]
    wvv = wvT.rearrange("(n p) j -> n p j", p=P)       # [8,128,512]
    wov = woT.rearrange("(n p) o -> n p o", p=P)       # [4,128,1024]
    outv = outT.rearrange("(n p) t -> n p t", p=P)     # [8,128,2048]

    inv8 = 1.0 / float(np.sqrt(DH))

    with tile.TileContext(nc) as tc:
        with (
            tc.tile_pool(name="persist", bufs=1) as persist,
            tc.tile_pool(name="ptpool", bufs=3) as ptpool,
            tc.tile_pool(name="small", bufs=1) as small,
            tc.tile_pool(name="psS", bufs=1, space="PSUM") as psS,
            tc.tile_pool(name="psY", bufs=1, space="PSUM") as psY,
        ):
            # ---- persistent SBUF tensors ----
            x_sb = persist.tile([P, NDT, T], BF16)        # whole xT
            qt = [persist.tile([P, NJT, TQ], BF16, name=f"qt{i}")
                  for i in range(NTT)]
            kt = [persist.tile([P, NJT, TQ], BF16, name=f"kt{i}")
                  for i in range(NTT)]
            vt = [persist.tile([P, TQ // P, HPC, VW], BF16, name=f"vt{i}")
                  for i in range(NTT)]
            yt = [persist.tile([P, NJT, TQ], BF16, name=f"yt{i}")
                  for i in range(NTT)]
            mask_sb = persist.tile([P, 1, P], BF16)
            ones_bf = persist.tile([1, P], BF16)
            wqk_sb = persist.tile([P, NDT, 2 * JJ], BF16)
            wv_sb = persist.tile([P, NDT, JJ], BF16)
            wo_sb = persist.tile([P, NJT, D], BF16)

            for i in range(NTT):
                nc.any.memset(vt[i][:, :, :, DH : DH + 1], 1.0)
            nc.any.memset(ones_bf[:], 1.0)

            # ---- input DMAs, in consumption order ----
            nc.sync.dma_start(out=mask_sb[:, 0, :], in_=maskd[:, :])
            for dt_ in range(NDT):
                nc.sync.dma_start(out=wqk_sb[:, dt_, :], in_=wqkv[dt_])
            for dt_ in range(NDT):
                nc.sync.dma_start(out=x_sb[:, dt_, :], in_=xTv[dt_])
            for dt_ in range(NDT):
                nc.sync.dma_start(out=wv_sb[:, dt_, :], in_=wvv[dt_])
            for it in range(NJT):
                nc.sync.dma_start(out=wo_sb[:, it, :], in_=wov[it])

            # ---- stage 1: QKV projection for one t-tile (generator:
            #      yields once per emitted matmul) ----
            def s1_stream(ti):
                tsl = slice(ti * TQ, (ti + 1) * TQ)
                for gi in range(12):
                    if gi < 8:
                        o_sb = qt[ti] if gi < 4 else kt[ti]
                        jt = gi % 4
                        joff = (0 if gi < 4 else JJ) + jt * P
                        ps = psS.tile([P, TQ], F32, tag="mm", bufs=2)
                        for dt_ in range(NDT):
                            nc.tensor.matmul(
                                ps[:],
                                lhsT=wqk_sb[:, dt_, joff : joff + P],
                                rhs=x_sb[:, dt_, tsl],
                                start=(dt_ == 0),
                                stop=(dt_ == NDT - 1),
                            )
                            if dt_ < NDT - 1:
                                yield
                        nc.vector.tensor_copy(o_sb[:, jt, :], ps[:])
                        yield
                    else:
                        tsub = gi - 8
                        ssl = slice(ti * TQ + tsub * P,
                                    ti * TQ + (tsub + 1) * P)
                        ps = psS.tile([P, JJ], F32, tag="mm", bufs=2)
                        for dt_ in range(NDT):
                            nc.tensor.matmul(
                                ps[:],
                                lhsT=x_sb[:, dt_, ssl],
                                rhs=wv_sb[:, dt_, :],
                                start=(dt_ == 0),
                                stop=(dt_ == NDT - 1),
                            )
                            if dt_ < NDT - 1:
                                yield
                        nc.vector.tensor_copy(
                            vt[ti][:, tsub, :, 0:DH],
                            ps[:].rearrange("p (h i) -> p h i", h=HPC),
                        )
                        yield

            # ---- stage 3: output projection (generator, one yield/mm) ----
            def s3_stream(tis):
                for ti in tis:
                    tsl = slice(ti * TQ, (ti + 1) * TQ)
                    for ot in range(NOT_):
                        osl = slice(ot * P, (ot + 1) * P)
                        ps = psS.tile([P, TQ], F32, tag="mm", bufs=2)
                        for it in range(NJT):
                            nc.tensor.matmul(
                                ps[:],
                                lhsT=wo_sb[:, it, osl],
                                rhs=yt[ti][:, it, :],
                                start=(it == 0),
                                stop=(it == NJT - 1),
                            )
                            if it < NJT - 1:
                                yield
                        o_sb = small.tile([P, TQ], BF16, tag="ostage",
                                          bufs=3)
                        nc.vector.tensor_copy(o_sb[:], ps[:])
                        nc.sync.dma_start(out=outv[ot][:, tsl], in_=o_sb[:])
                        yield

            def fill(state, n):
                state[1] += n
                while state[1] >= 1.0:
                    state[1] -= 1.0
                    if next(state[0], "done") == "done":
                        state[1] = -1e9  # stream dry
                        break

            # ---- stage 2: attention for one (qi, g) head pair ----
            def s2_block(qi, g, state, rate):
                n_full = 4 * qi
                nkt = n_full + 4

                def emit_S(kt_i):
                    m = kt_i - n_full
                    qoff = 0 if m < 0 else m * P
                    ksl = slice((kt_i % 4) * P, (kt_i % 4 + 1) * P)
                    s2 = psS.tile([P, 2, TQ], F32, tag="att", bufs=2)
                    for hh in range(2):
                        hsl = slice(hh * DH, (hh + 1) * DH)
                        nc.tensor.matmul(
                            s2[:, hh, qoff:TQ],
                            lhsT=kt[kt_i // 4][hsl, g, ksl],
                            rhs=qt[qi][hsl, g, qoff:TQ],
                            start=True,
                            stop=True,
                        )
                    pt2 = ptpool.tile([P, 2, TQ], BF16, tag="pt")
                    nc.scalar.activation(
                        pt2[:, :, qoff:TQ], s2[:, :, qoff:TQ],
                        mybir.ActivationFunctionType.Exp,
                        scale=inv8,
                    )
                    if m >= 0:  # diagonal block: zero where k > q
                        nc.vector.tensor_tensor(
                            pt2[:, :, qoff : qoff + P],
                            pt2[:, :, qoff : qoff + P],
                            mask_sb[:].to_broadcast([P, 2, P]),
                            mybir.AluOpType.mult,
                        )
                    return pt2, qoff

                def emit_PV(kt_i, pt2, qoff):
                    for hh in range(2):
                        nc.tensor.matmul(
                            y2[:, hh, qoff:TQ],
                            lhsT=vt[kt_i // 4][:, kt_i % 4, 2 * g + hh,
                                              0 : DH + 1],
                            rhs=pt2[:, hh, qoff:TQ],
                            start=(kt_i == 0),
                            stop=(kt_i == nkt - 1),
                        )

                y2 = psY.tile([DH + 1, 2, TQ], F32, tag="y", bufs=1,
                              name=f"y2_{qi}_{g}")
                prev = None
                for kt_i in range(nkt):
                    cur = emit_S(kt_i)
                    if prev is not None:
                        emit_PV(kt_i - 1, *prev)
                    fill(state, rate)
                    prev = cur
                emit_PV(nkt - 1, *prev)
                # pull rowsum + unnormalized y out of PSUM (frees y2 early)
                rs_sb = small.tile([1, 2, TQ], F32, tag="rs_sb", bufs=2)
                ystage = small.tile([DH, 2, TQ], BF16, tag="ystage", bufs=2)
                for hh in range(2):
                    nc.vector.tensor_copy(
                        rs_sb[0:1, hh, :], y2[DH : DH + 1, hh, :]
                    )
                    nc.vector.tensor_copy(
                        ystage[:, hh, :], y2[0:DH, hh, :]
                    )
                fill(state, rate)
                # 1/rowsum, broadcast to 64 partitions via PE, multiply
                recipf = small.tile([1, 2, TQ], F32, tag="recipf", bufs=2)
                nc.vector.reciprocal_approx_fast(recipf[:], rs_sb[:])
                rbf = small.tile([1, 2, TQ], BF16, tag="rbf", bufs=2)
                nc.vector.tensor_copy(rbf[:], recipf[:])
                for hh in range(2):
                    bc_ps = psS.tile([DH, TQ], F32, tag="mm", bufs=2)
                    nc.tensor.matmul(
                        bc_ps[:],
                        lhsT=ones_bf[0:1, 0:DH],
                        rhs=rbf[0:1, hh, :],
                        start=True,
                        stop=True,
                    )
                    bcb = small.tile([DH, TQ], BF16, tag="bcb", bufs=2)
                    nc.vector.tensor_copy(bcb[:], bc_ps[:])
                    nc.vector.tensor_tensor(
                        yt[qi][hh * DH : (hh + 1) * DH, g, :],
                        ystage[:, hh, :],
                        bcb[:],
                        mybir.AluOpType.mult,
                    )

            # ---- emission schedule ----
            for _ in s1_stream(0):
                pass
            for qi in range(NTT):
                iters = 4 * (4 * qi + 4)
                if qi + 1 < NTT:
                    state = [s1_stream(qi + 1), 0.0]
                else:
                    state = [s3_stream([0, 1, 2]), 0.0]
                rate = 100.0 / iters
                for g in range(NJT):
                    s2_block(qi, g, state, rate)
                for _ in state[0]:  # drain leftovers
                    pass
            for _ in s3_stream([NTT - 1]):
                pass

    nc.compile()
    return nc


def _make_mask():
    k = np.arange(P)[:, None]
    j = np.arange(P)[None, :]
    return (j >= k).astype(np.float32)


def make_in_maps(x, Wq, Wk, Wv, Wo):
    import ml_dtypes

    bf = ml_dtypes.bfloat16
    mask = _make_mask().astype(bf)
    x = np.asarray(x, np.float32)
    Wq, Wk, Wv, Wo = (np.asarray(w, np.float32) for w in (Wq, Wk, Wv, Wo))
    in_maps = []
    for c in range(NCORES):
        b, hg = c // 2, c % 2
        sl = slice(hg * JJ, (hg + 1) * JJ)
        wqk = np.concatenate(
            [np.ascontiguousarray(Wq[sl].T), np.ascontiguousarray(Wk[sl].T)],
            axis=1,
        )
        in_maps.append({
            "xT": np.ascontiguousarray(x[b].T).astype(bf),
            "wqkT": np.ascontiguousarray(wqk).astype(bf),
            "wvT": np.ascontiguousarray(Wv[sl].T).astype(bf),
            "woT": np.ascontiguousarray(Wo[:, sl].T).astype(bf),
            "mask": mask,
        })
    return in_maps


def gather_output(results):
    out = np.zeros((B, T, D), np.float32)
    for c in range(NCORES):
        out[c // 2] += results[c]["outT"].T.astype(np.float32)
    return out


def kernel(x, Wq, Wk, Wv, Wo):
    nc = build_program()
    in_maps = make_in_maps(x, Wq, Wk, Wv, Wo)
    res = run_bass_kernel_spmd(nc, in_maps, list(range(NCORES)))
    return gather_output(res.results)


if __name__ == "__main__":
    rng = np.random.default_rng(0)
    xs = [rng.standard_normal(s, dtype=np.float32) for s in
          [(B, T, D), (D, D), (D, D), (D, D), (D, D)]]
    out = kernel(*xs)
    print(out.shape, out.dtype)


# revision 19
# speedup vs baseline: 1.3159x; 1.0629x over previous
"""Causal self-attention on 8 Trainium2 NeuronCores.

Problem: x[4,2048,1024], Wq/Wk/Wv/Wo[1024,1024], H=16 heads, dh=64.
    q,k,v = x@W{q,k,v}.T ; per-head causal softmax(q k^T/8) v ; out = y@Wo.T

Sharding (hybrid data+tensor parallel over 8 cores):
  core c -> (batch b = c//2, head-group hg = c%2 of 8 heads = 512 dims).
  Each core computes a partial output out_c[b] = y_hg @ Wo[:, hg].T ; the
  host sums the two bf16 partials per batch in f32 (the Wo all-reduce).

Per-core kernel dataflow (layouts avoid any on-device transposes):
  xT[1024,2048] (=x[b].T), wqkT[1024,1024] (=[Wq[hg].T | Wk[hg].T]),
  wvT[1024,512], woT[512,1024] (=Wo[:,hg].T)
  stage 1 (per 512-col t-tile): QT[j,t], KT[j,t] via matmul(lhsT=w, rhs=xT);
           V[t,i] via matmul(lhsT=xT slice, rhs=wvT); a ones column per head
           appended to V so the PV matmul emits softmax row-sums for free.
  stage 2: per head pair g, per 128-wide k-tile: the two heads' S^T[k,q]
           matmuls row-tiled on the PE (K=64 at tile rows 0/64, concurrent)
           into adjacent PSUM banks; one fused exp(S/8) over both banks on
           ACT (bf16 out); causal handling: ops on diagonal k-tiles trimmed
           to q >= 128m plus one [128,128] triangle-mask multiply on the
           diagonal block; yT_h[65,q] += V'_h[k,65]^T P^T_h (row 64 =
           softmax denominator).
           All (g, kt) iterations of all q-tiles form ONE software-pipelined
           emission stream: PV(i) is emitted two iterations behind S(i) (so
           exp latency and the per-block PSUM-release chains are covered by
           independent work), and stage-1/stage-3 filler matmuls (yielded
           one at a time by generators, ~1-6 per iteration by phase) keep
           the in-order PE queue dense -> HAM clock-gate stays at 2.4GHz.
           Normalize per g: rowsum row + unnormalized y copied out of PSUM
           early (frees the y bank), fast reciprocal -> bf16 -> K=1
           ones-matmul broadcast to 64 partitions -> multiply.
  stage 3: outT[o,t] = matmul(lhsT=woT, rhs=yT) -> bf16 -> DMA out.

  Startup: DMAs are emitted in consumption order ([wqk dt | x-lo dt]
  pairs first) so the first projection's accumulation chain can start
  after ~0.5MB arrives; ~96 warmup matmuls on the mask tile bridge the
  initial DMA wait so the PE hits the first real matmul at full clock.

Precision: matmul operands bf16, PSUM accumulation fp32, softmax
reciprocal via fast-approx (f32, SBUF-staged: the custom DVE op reading
PSUM directly is silently wrong on HW) cast to bf16. exp needs no
max-subtraction: att ~ N(0,1) here, |att| < ~7, exp is safe in fp32.
"""

import sys

import numpy as np

sys.path.insert(0, "/opt/trn_rl_repo")

import concourse.bass as bass  # noqa: F401
from concourse import bacc
import concourse.mybir as mybir
import concourse.tile as tile
from concourse.bass_utils import run_bass_kernel_spmd

B, T, D, H, DH = 4, 2048, 1024, 16, 64
NCORES = 8
HPC = 8                 # heads per core
JJ = HPC * DH           # 512: per-core qkv head dims
P = 128
TQ = 512                # attention q tile (free dim of S^T matmul)
TK = 128                # attention k tile (partition dim of S^T)
NDT = D // P            # 8 d-tiles (contraction for stage 1)
NJT = JJ // P           # 4 j-tiles (head-pair tiles)
NTT = T // TQ           # 4 t-tiles of 512
NKT = T // TK           # 16 k-tiles of 128
NOT_ = D // P           # 8 output row tiles (stage 3)
VW = 66                 # V row width: 64 dh + 1 ones + 1 pad
TH = T // 2             # 1024: x half-tile width (2KB DMA lines)
F32 = mybir.dt.float32
BF16 = mybir.dt.bfloat16


def build_program():
    nc = bacc.Bacc()
    xT = nc.dram_tensor("xT", [D, T], BF16, kind="ExternalInput")
    wqkT = nc.dram_tensor("wqkT", [D, 2 * JJ], BF16, kind="ExternalInput")
    wvT = nc.dram_tensor("wvT", [D, JJ], BF16, kind="ExternalInput")
    woT = nc.dram_tensor("woT", [JJ, D], BF16, kind="ExternalInput")
    maskd = nc.dram_tensor("mask", [P, P], BF16, kind="ExternalInput")
    outT = nc.dram_tensor("outT", [D, T], BF16, kind="ExternalOutput")

    xTv = xT.rearrange("(n p) t -> n p t", p=P)        # [8,128,2048]
    wqkv = wqkT.rearrange("(n p) j -> n p j", p=P)     # [8,128,1024]
    wvv = wvT.rearrange("(n p) j -> n p j", p=P)       # [8,128,512]
    wov = woT.rearrange("(n p) o -> n p o", p=P)       # [4,128,1024]
    outv = outT.rearrange("(n p) t -> n p t", p=P)     # [8,128,2048]

    inv8 = 1.0 / float(np.sqrt(DH))

    with tile.TileContext(nc) as tc:
        with (
            tc.tile_pool(name="persist", bufs=1) as persist,
            tc.tile_pool(name="ptpool", bufs=4) as ptpool,
            tc.tile_pool(name="small", bufs=1) as small,
            tc.tile_pool(name="psS", bufs=1, space="PSUM") as psS,
            tc.tile_pool(name="psY", bufs=1, space="PSUM") as psY,
        ):
            # ---- persistent SBUF tensors ----
            x_sb = [persist.tile([P, NDT, TH], BF16, name=f"x{i}")
                    for i in range(2)]
            qt = [persist.tile([P, NJT, TQ], BF16, name=f"qt{i}")
                  for i in range(NTT)]
            kt = [persist.tile([P, NJT, TQ], BF16, name=f"kt{i}")
                  for i in range(NTT)]
            vt = [persist.tile([P, TQ // P, HPC, VW], BF16, name=f"vt{i}")
                  for i in range(NTT)]
            yt = [persist.tile([P, NJT, TQ], BF16, name=f"yt{i}")
                  for i in range(NTT)]
            mask_sb = persist.tile([P, 1, P], BF16)
            ones_bf = persist.tile([1, P], BF16)
            wqk_sb = persist.tile([P, NDT, 2 * JJ], BF16)
            wv_sb = persist.tile([P, NDT, JJ], BF16)
            wo_sb = persist.tile([P, NJT, D], BF16)

            def xap(dt_, tlo, thi):  # x slice [P, tlo:thi] within one half
                h = tlo // TH
                assert (thi - 1) // TH == h
                return x_sb[h][:, dt_, tlo - h * TH : thi - h * TH]

            for i in range(NTT):
                nc.any.memset(vt[i][:, :, :, DH : DH + 1], 1.0)
            nc.any.memset(ones_bf[:], 1.0)

            # ---- input DMAs, in consumption order ----
            nc.sync.dma_start(out=mask_sb[:, 0, :], in_=maskd[:, :])
            for dt_ in range(NDT):
                nc.sync.dma_start(out=wqk_sb[:, dt_, :], in_=wqkv[dt_])
                nc.sync.dma_start(out=x_sb[0][:, dt_, :],
                                  in_=xTv[dt_][:, 0:TH])
            for dt_ in range(NDT):
                nc.sync.dma_start(out=wv_sb[:, dt_, :], in_=wvv[dt_])
            for dt_ in range(NDT):
                nc.sync.dma_start(out=x_sb[1][:, dt_, :],
                                  in_=xTv[dt_][:, TH:T])
            for it in range(NJT):
                nc.sync.dma_start(out=wo_sb[:, it, :], in_=wov[it])

            # ---- PE warmup: bridge the initial DMA wait at full clock ----
            warm = psS.tile([P, 2, TQ], F32, tag="att", bufs=2)
            for i in range(96):
                nc.tensor.matmul(
                    warm[:, 0, 0:P],
                    lhsT=mask_sb[:, 0, :],
                    rhs=mask_sb[:, 0, :],
                    start=True,
                    stop=True,
                )

            # ---- stage 1: QKV projection for one t-tile (generator:
            #      yields once per emitted matmul) ----
            def s1_stream(ti):
                tlo = ti * TQ
                for gi in range(12):
                    if gi < 8:
                        o_sb = qt[ti] if gi < 4 else kt[ti]
                        jt = gi % 4
                        joff = (0 if gi < 4 else JJ) + jt * P
                        ps = psS.tile([P, TQ], F32, tag="mm", bufs=2)
                        for dt_ in range(NDT):
                            nc.tensor.matmul(
                                ps[:],
                                lhsT=wqk_sb[:, dt_, joff : joff + P],
                                rhs=xap(dt_, tlo, tlo + TQ),
                                start=(dt_ == 0),
                                stop=(dt_ == NDT - 1),
                            )
                            if dt_ < NDT - 1:
                                yield
                        nc.vector.tensor_copy(o_sb[:, jt, :], ps[:])
                        yield
                    else:
                        tsub = gi - 8
                        ps = psS.tile([P, JJ], F32, tag="mm", bufs=2)
                        for dt_ in range(NDT):
                            nc.tensor.matmul(
                                ps[:],
                                lhsT=xap(dt_, tlo + tsub * P,
                                         tlo + (tsub + 1) * P),
                                rhs=wv_sb[:, dt_, :],
                                start=(dt_ == 0),
                                stop=(dt_ == NDT - 1),
                            )
                            if dt_ < NDT - 1:
                                yield
                        nc.vector.tensor_copy(
                            vt[ti][:, tsub, :, 0:DH],
                            ps[:].rearrange("p (h i) -> p h i", h=HPC),
                        )
                        yield

            # ---- stage 3: output projection (generator, one yield/mm) ----
            def s3_stream(tis):
                for ti in tis:
                    tsl = slice(ti * TQ, (ti + 1) * TQ)
                    for ot in range(NOT_):
                        osl = slice(ot * P, (ot + 1) * P)
                        ps = psS.tile([P, TQ], F32, tag="mm", bufs=2)
                        for it in range(NJT):
                            nc.tensor.matmul(
                                ps[:],
                                lhsT=wo_sb[:, it, osl],
                                rhs=yt[ti][:, it, :],
                                start=(it == 0),
                                stop=(it == NJT - 1),
                            )
                            if it < NJT - 1:
                                yield
                        o_sb = small.tile([P, TQ], BF16, tag="ostage",
                                          bufs=3)
                        nc.vector.tensor_copy(o_sb[:], ps[:])
                        nc.sync.dma_start(out=outv[ot][:, tsl], in_=o_sb[:])
                        yield

            def fill(state, n):
                state[1] += n
                while state[1] >= 1.0:
                    state[1] -= 1.0
                    if next(state[0], "done") == "done":
                        state[1] = -1e9  # stream dry
                        break

            # ---- stage 2 pieces ----
            def emit_S(qi, g, kt_i):
                n_full = 4 * qi
                m = kt_i - n_full
                qoff = 0 if m < 0 else m * P
                ksl = slice((kt_i % 4) * P, (kt_i % 4 + 1) * P)
                s2 = psS.tile([P, 2, TQ], F32, tag="att", bufs=2)
                for hh in range(2):
                    hsl = slice(hh * DH, (hh + 1) * DH)
                    nc.tensor.matmul(
                        s2[:, hh, qoff:TQ],
                        lhsT=kt[kt_i // 4][hsl, g, ksl],
                        rhs=qt[qi][hsl, g, qoff:TQ],
                        start=True,
                        stop=True,
                    )
                pt2 = ptpool.tile([P, 2, TQ], BF16, tag="pt")
                nc.scalar.activation(
                    pt2[:, :, qoff:TQ], s2[:, :, qoff:TQ],
                    mybir.ActivationFunctionType.Exp,
                    scale=inv8,
                )
                if m >= 0:  # diagonal block: zero where k > q
                    nc.vector.tensor_tensor(
                        pt2[:, :, qoff : qoff + P],
                        pt2[:, :, qoff : qoff + P],
                        mask_sb[:].to_broadcast([P, 2, P]),
                        mybir.AluOpType.mult,
                    )
                return pt2, qoff

            def emit_PV(qi, g, kt_i, y2, pt2, qoff):
                nkt = 4 * qi + 4
                for hh in range(2):
                    nc.tensor.matmul(
                        y2[:, hh, qoff:TQ],
                        lhsT=vt[kt_i // 4][:, kt_i % 4, 2 * g + hh,
                                          0 : DH + 1],
                        rhs=pt2[:, hh, qoff:TQ],
                        start=(kt_i == 0),
                        stop=(kt_i == nkt - 1),
                    )

            def emit_norm(qi, g, y2):
                # pull rowsum + unnormalized y out of PSUM (frees y2)
                rs_sb = small.tile([1, 2, TQ], F32, tag="rs_sb", bufs=2)
                ystage = small.tile([DH, 2, TQ], BF16, tag="ystage", bufs=2)
                for hh in range(2):
                    nc.vector.tensor_copy(
                        rs_sb[0:1, hh, :], y2[DH : DH + 1, hh, :]
                    )
                    nc.vector.tensor_copy(ystage[:, hh, :], y2[0:DH, hh, :])
                # 1/rowsum, broadcast to 64 partitions via PE, multiply
                recipf = small.tile([1, 2, TQ], F32, tag="recipf", bufs=2)
                nc.vector.reciprocal_approx_fast(recipf[:], rs_sb[:])
                rbf = small.tile([1, 2, TQ], BF16, tag="rbf", bufs=2)
                nc.vector.tensor_copy(rbf[:], recipf[:])
                for hh in range(2):
                    bc_ps = psS.tile([DH, TQ], F32, tag="mm", bufs=2)
                    nc.tensor.matmul(
                        bc_ps[:],
                        lhsT=ones_bf[0:1, 0:DH],
                        rhs=rbf[0:1, hh, :],
                        start=True,
                        stop=True,
                    )
                    bcb = small.tile([DH, TQ], BF16, tag="bcb", bufs=2)
                    nc.vector.tensor_copy(bcb[:], bc_ps[:])
                    nc.vector.tensor_tensor(
                        yt[qi][hh * DH : (hh + 1) * DH, g, :],
                        ystage[:, hh, :],
                        bcb[:],
                        mybir.AluOpType.mult,
                    )

            # ---- emission schedule: one global pipelined stream ----
            for _ in s1_stream(0):
                pass

            pend = []          # up to 2 in-flight (qi, g, kt_i, y2, pt, qoff)
            y2cur = None

            def retire_one():
                qi_, g_, kt_, y2_, pt_, qo_ = pend.pop(0)
                emit_PV(qi_, g_, kt_, y2_, pt_, qo_)
                if kt_ == 4 * qi_ + 3:  # last kt of block -> normalize
                    emit_norm(qi_, g_, y2_)

            for qi in range(NTT):
                nkt = 4 * qi + 4
                iters = 4 * nkt
                if qi + 1 < NTT:
                    state = [s1_stream(qi + 1), 0.0]
                else:
                    state = [s3_stream([0, 1, 2]), 0.0]
                rate = 100.0 / iters
                for g in range(NJT):
                    for kt_i in range(nkt):
                        if kt_i == 0:
                            y2cur = psY.tile([DH + 1, 2, TQ], F32, tag="y",
                                             bufs=1, name=f"y2_{qi}_{g}")
                        cur = (qi, g, kt_i, y2cur) + emit_S(qi, g, kt_i)
                        if len(pend) >= 2:
                            retire_one()
                        fill(state, rate)
                        pend.append(cur)
                for _ in state[0]:  # qi boundary: drain leftover fillers
                    pass
            while pend:
                retire_one()
            for _ in s3_stream([NTT - 1]):
                pass

    nc.compile()
    return nc


def _make_mask():
    k = np.arange(P)[:, None]
    j = np.arange(P)[None, :]
    return (j >= k).astype(np.float32)


def make_in_maps(x, Wq, Wk, Wv, Wo):
    import ml_dtypes

    bf = ml_dtypes.bfloat16
    mask = _make_mask().astype(bf)
    x = np.asarray(x, np.float32)
    Wq, Wk, Wv, Wo = (np.asarray(w, np.float32) for w in (Wq, Wk, Wv, Wo))
    in_maps = []
    for c in range(NCORES):
        b, hg = c // 2, c % 2
        sl = slice(hg * JJ, (hg + 1) * JJ)
        wqk = np.concatenate(
            [np.ascontiguousarray(Wq[sl].T), np.ascontiguousarray(Wk[sl].T)],
            axis=1,
        )
        in_maps.append({
            "xT": np.ascontiguousarray(x[b].T).astype(bf),
            "wqkT": np.ascontiguousarray(wqk).astype(bf),
            "wvT": np.ascontiguousarray(Wv[sl].T).astype(bf),
            "woT": np.ascontiguousarray(Wo[:, sl].T).astype(bf),
            "mask": mask,
        })
    return in_maps


def gather_output(results):
    out = np.zeros((B, T, D), np.float32)
    for c in range(NCORES):
        out[c // 2] += results[c]["outT"].T.astype(np.float32)
    return out


def kernel(x, Wq, Wk, Wv, Wo):
    nc = build_program()
    in_maps = make_in_maps(x, Wq, Wk, Wv, Wo)
    res = run_bass_kernel_spmd(nc, in_maps, list(range(NCORES)))
    return gather_output(res.results)


if __name__ == "__main__":
    rng = np.random.default_rng(0)
    xs = [rng.standard_normal(s, dtype=np.float32) for s in
          [(B, T, D), (D, D), (D, D), (D, D), (D, D)]]
    out = kernel(*xs)
    print(out.shape, out.dtype)
